# revision 1
# baseline (speedup 1.0000x reference)
"""TRN2 Bass kernel for nn_CrossAttentionScorer.

The module collapses algebraically: seq-len is 1, so softmax over the single
attention score is identically 1.0 and the attention output equals `v`
exactly — the whole q/k path is dead code. The remaining computation is

    z   = layernorm(candidate)                 (ln weight folded into W)
    out = relu(z @ W.T + bh) @ sign_vec + b2

with W = (|w2| * (w1 @ wo @ wv) * ln_w) folded on the host and sign(w2)
handled by permuting FF columns so the final dot product becomes
sum(relu(pos block)) - sum(relu(neg block)).

Engine split per 128-row tile (no-bias fast path):
  relu(rsig*(xc@W)) = rsig*relu(xc@W) since rsig>0, so the variance/rsqrt
  chain is off the critical path. ScalarE does the front-end (row sum,
  center), TensorE transposes xc and runs fp32r matmuls, VectorE does the
  fp32r rounding casts + fused relu-and-accumulate + the final rsig scale.

Data parallel over 8 NeuronCores: batch 32768 -> 8 x 4096 rows; weights
replicated. fp32r matmuls: full PE rate at N=512, ~1.5e-4 rel err.
"""

import numpy as np

_B, _D, _FF = 32768, 1024, 2048
_NC = 8
_P = 128
_SHARD = _B // _NC     # 4096 rows per core
_NTILE = _SHARD // _P  # 32 tiles of 128 rows
_KC = _D // _P         # 8 contraction chunks
_NTW = 512             # matmul moving free size (one PSUM bank of fp32)
_NFT = _FF // _NTW     # 4 ff tiles

_program_cache = {}


def _build_program(P0: int, has_bias: bool, ntile: int = _NTILE):
    import concourse.bacc as bacc
    import concourse.mybir as mybir
    import concourse.tile as tile
    from concourse import masks
    from contextlib import ExitStack

    f32 = mybir.dt.float32
    f32r = mybir.dt.float32r
    AF = mybir.ActivationFunctionType
    AX = mybir.AxisListType
    ALU = mybir.AluOpType

    shard = ntile * _P
    nc = bacc.Bacc("TRN2", target_bir_lowering=False, debug=False)
    x_d = nc.dram_tensor("x", [shard, _D], f32, kind="ExternalInput")
    wt_d = nc.dram_tensor("wt", [_D, _FF], f32, kind="ExternalInput")
    if has_bias:
        bh_d = nc.dram_tensor("bh", [1, _FF], f32, kind="ExternalInput")
    o_d = nc.dram_tensor("o", [shard, 1], f32, kind="ExternalOutput")

    inv_d = 1.0 / _D

    # pos/neg split per ff tile (pos block is a prefix after host permutation)
    # acc columns: pos slices first (prefix), then neg slices; the combine is
    # sum(pos cols) - sum(neg cols), scaled by rsig.
    slices = []  # (nt, lo, hi, sign)
    for nt in range(_NFT):
        lo, hi = nt * _NTW, (nt + 1) * _NTW
        npos = min(max(P0 - lo, 0), _NTW)
        if npos > 0:
            slices.append((nt, 0, npos, 1.0))
        if npos < _NTW:
            slices.append((nt, npos, _NTW, -1.0))
    ncol = len(slices)
    kp = sum(1 for s in slices if s[3] > 0)   # pos cols are a prefix

    with tile.TileContext(nc) as tc, ExitStack() as ctx:
        const = ctx.enter_context(tc.tile_pool(name="const", bufs=1))
        wpool = ctx.enter_context(tc.tile_pool(name="w", bufs=1))
        wstage = ctx.enter_context(tc.tile_pool(name="wstage", bufs=3))
        xpool = ctx.enter_context(tc.tile_pool(name="xp", bufs=8))
        zpool = ctx.enter_context(tc.tile_pool(name="zp", bufs=3))
        ztpool = ctx.enter_context(tc.tile_pool(name="ztp", bufs=3))
        dpool = ctx.enter_context(tc.tile_pool(name="dump", bufs=1))
        spool = ctx.enter_context(tc.tile_pool(name="sp", bufs=6))
        apool = ctx.enter_context(tc.tile_pool(name="acc", bufs=3))
        opool = ctx.enter_context(tc.tile_pool(name="op", bufs=4))
        pst = ctx.enter_context(
            tc.tile_pool(name="pst", bufs=2 if has_bias else 3, space="PSUM"))
        psm = ctx.enter_context(
            tc.tile_pool(name="psm", bufs=3 if has_bias else 4, space="PSUM"))

        ident32 = const.tile([_P, _P], f32)
        masks.make_identity(nc, ident32[:])
        ident = const.tile([_P, _P], f32r)
        nc.vector.tensor_copy(ident[:], ident32[:])
        epsT = const.tile([_P, 1], f32)
        nc.gpsimd.memset(epsT[:], 1e-5)

        # weights: DMA fp32, round to fp32r (one-time). nt-outer order so the
        # first ff-tile's 8 chunks are ready first; casts alternate DVE/ACT so
        # neither engine is blocked for long at startup.
        wtr = wpool.tile([_P, _KC * _FF], f32r)
        wt_r = wt_d.rearrange("(kc p) n -> kc p n", p=_P)

        def load_weights(nt):
            for kc in range(_KC):
                st = wstage.tile([_P, _NTW], f32)
                nc.sync.dma_start(st[:], wt_r[kc][:, nt * _NTW:(nt + 1) * _NTW])
                dst = wtr[:, kc * _FF + nt * _NTW: kc * _FF + (nt + 1) * _NTW]
                if kc % 2 == 0:
                    nc.vector.tensor_copy(dst, st[:])
                else:
                    nc.scalar.activation(dst, st[:], AF.Identity)

        if has_bias:
            bh32 = const.tile([1, _FF], f32)
            bhr = const.tile([1, _FF], f32r)
            nc.sync.dma_start(bh32[:], bh_d[:, :])
            nc.vector.tensor_copy(bhr[:], bh32[:])

        x_r = x_d.rearrange("(t p) d -> t p d", p=_P)
        o_r = o_d.rearrange("(t p) one -> t p one", p=_P)

        # Software-pipelined emission: F(t) = load/stats/center/transpose/cast,
        # B(t) = matmuls/relu-accum/combine/store. Emitting F(t+1) before B(t)
        # keeps each engine's FIFO free of head-of-line blocking: ScalarE only
        # runs front-end work, VectorE's MM-dependent ops never delay the next
        # tile's inputs, and TensorE alternates transp(t+1) | MM(t) gap-free.
        state = {}

        xtiles = {}

        def load(t):
            x = xpool.tile([_P, _D], f32)
            nc.gpsimd.dma_start(x[:], x_r[t])
            xtiles[t] = x

        def front(t):
            x = xtiles.pop(t)

            # VectorE: -mean in one fused op (dump write is discarded)
            dumpv = dpool.tile([_P, _D], f32, tag="dumpv")
            negmu = spool.tile([_P, 1], f32)
            nc.vector.tensor_scalar(
                out=dumpv[:], in0=x[:], scalar1=-inv_d, scalar2=None,
                op0=ALU.mult, op1=ALU.add, accum_out=negmu[:])
            # ScalarE critical path: center (fp32r for the transposes)
            xc = zpool.tile([_P, _D], f32r)
            nc.scalar.activation(xc[:], x[:], AF.Identity, bias=negmu[:])

            # PE transpose xc -> zt; ScalarE does the PSUM->SBUF drain
            zt = ztpool.tile([_P, _D], f32r)
            for half in range(2):
                tp = pst.tile([_P, _NTW], f32r)
                for j in range(4):
                    c = half * 4 + j
                    nc.tensor.transpose(tp[:, j * _P:(j + 1) * _P],
                                        xc[:, c * _P:(c + 1) * _P], ident[:])
                nc.scalar.activation(zt[:, half * _NTW:(half + 1) * _NTW],
                                     tp[:], AF.Identity)

            # variance chain (only gates the final per-row scale; emitted after
            # the casts so it never delays what TensorE waits on)
            dump2 = dpool.tile([_P, _D], f32, tag="dump2")
            s2 = spool.tile([_P, 1], f32)
            nc.scalar.activation(dump2[:], xc[:], AF.Square, accum_out=s2[:])
            v = spool.tile([_P, 1], f32)
            nc.scalar.activation(v[:], s2[:], AF.Identity, scale=inv_d, bias=epsT[:])
            rv = spool.tile([_P, 1], f32)
            nc.vector.reciprocal(rv[:], v[:])
            rsig = spool.tile([_P, 1], f32)
            nc.scalar.activation(rsig[:], rv[:], AF.Sqrt)

            st = {"zt": zt, "rsig": rsig}
            if has_bias:
                # sqrt(var+eps) = v * rsig; transposed below for the rank-1
                # bias matmul (bh enters pre-relu as (1/rsig)_b * bh_ff)
                sqv = spool.tile([_P, 1], f32)
                nc.vector.tensor_mul(sqv[:], v[:], rsig[:])
                tpb = pst.tile([_P, _P], f32, tag="tpb")
                nc.tensor.transpose(tpb[:], sqv[:].to_broadcast((_P, _P)), ident32[:])
                sqvr = ztpool.tile([1, _P], f32r, tag="sqvr")
                nc.vector.tensor_copy(sqvr[:], tpb[0:1, :])
                st["sqvr"] = sqvr
            state[t] = st

        def back(t):
            st = state.pop(t)
            zt, rsig = st["zt"], st["rsig"]
            acc = apool.tile([_P, ncol], f32)
            hdump = dpool.tile([_P, _NTW], f32, tag="hdump")
            col = 0
            for nt in range(_NFT):
                ps = psm.tile([_P, _NTW], f32)
                for kc in range(_KC):
                    nc.tensor.matmul(
                        ps[:], zt[:, kc * _P:(kc + 1) * _P],
                        wtr[:, kc * _FF + nt * _NTW: kc * _FF + (nt + 1) * _NTW],
                        start=(kc == 0),
                        stop=(kc == _KC - 1 and not has_bias))
                if has_bias:
                    nc.tensor.matmul(ps[:], st["sqvr"][:],
                                     bhr[:, nt * _NTW:(nt + 1) * _NTW],
                                     start=False, stop=True)
                for (snt, lo, hi, sgn) in slices:
                    if snt != nt:
                        continue
                    nc.vector.tensor_scalar(
                        out=hdump[:, lo:hi], in0=ps[:, lo:hi],
                        scalar1=0.0, scalar2=None,
                        op0=ALU.max, op1=ALU.add,
                        accum_out=acc[:, col:col + 1])
                    col += 1
            assert col == ncol

            # combine: out = rsig * (sum(pos cols) - sum(neg cols))
            o = opool.tile([_P, 1], f32)
            if 0 < kp < ncol:
                oP = spool.tile([_P, 1], f32, tag="oP")
                oN = spool.tile([_P, 1], f32, tag="oN")
                nc.vector.reduce_sum(oP[:], acc[:, 0:kp], axis=AX.X)
                nc.vector.reduce_sum(oN[:], acc[:, kp:ncol], axis=AX.X)
                S = spool.tile([_P, 1], f32, tag="S")
                nc.vector.tensor_sub(S[:], oP[:], oN[:])
            else:
                S = spool.tile([_P, 1], f32, tag="S")
                nc.vector.reduce_sum(S[:], acc[:, 0:ncol], axis=AX.X)
                if kp == 0:
                    nc.vector.tensor_scalar_mul(S[:], S[:], -1.0)
            nc.vector.tensor_mul(o[:], S[:], rsig[:])
            nc.sync.dma_start(o_r[t], o[:])

        # DMA prefetch runs 4 tiles ahead of the front-end stage, which runs
        # 1 tile ahead of the matmul/back-end stage. Weight casts for the
        # first ff-tile precede tile 0's front-end on the engine queues;
        # the rest follow it, so neither blocks the other at startup.
        for t in range(min(6, ntile)):
            load(t)
        load_weights(0)
        front(0)
        for nt in range(1, _NFT):
            load_weights(nt)
        for t in range(ntile):
            if t + 6 < ntile:
                load(t + 6)
            if t + 1 < ntile:
                front(t + 1)
            back(t)

    nc.compile()
    return nc


def _get_program(P0: int, has_bias: bool):
    key = (P0, has_bias)
    if key not in _program_cache:
        _program_cache[key] = _build_program(P0, has_bias)
    return _program_cache[key]


def _fold_weights(inputs):
    gd = lambda k: np.asarray(inputs[k], dtype=np.float64)
    wv, wo, w1, w2 = gd("wv"), gd("wo"), gd("w1"), gd("w2")
    bv, bo, b1, b2 = gd("bv"), gd("bo"), gd("b1"), gd("b2")
    lnw, lnb = gd("ln_kv_w"), gd("ln_kv_b")

    M = w1 @ wo @ wv                              # [FF, D]
    bias_h = M @ lnb + w1 @ (wo @ bv + bo) + b1   # [FF]
    We = M * lnw[None, :]                         # fold LN weight into columns

    w2v = w2.reshape(-1)                          # [FF]
    aw2 = np.abs(w2v)
    sgn = np.sign(w2v)
    perm = np.argsort(-sgn, kind="stable")        # +1 block, then 0, then -1
    P0 = int((sgn >= 0).sum())

    Wf = (We * aw2[:, None])[perm]                # [FF, D]
    bf = (bias_h * aw2)[perm]                     # [FF]

    Wt = np.ascontiguousarray(Wf.T).astype(np.float32)   # [D, FF]
    bh = bf.astype(np.float32)[None, :]                  # [1, FF]
    has_bias = bool(np.any(bh != 0.0))
    return Wt, bh, has_bias, P0, float(b2.reshape(-1)[0])


def kernel(run_opts=None, **inputs):
    """Full inputs in, full [B, 1] float32 output out. 8-core data parallel."""
    from concourse.bass_utils import run_bass_kernel_spmd

    x = np.ascontiguousarray(np.asarray(inputs["candidate_feature"],
                                        dtype=np.float32))
    assert x.shape == (_B, _D)

    Wt, bh, has_bias, P0, b2 = _fold_weights(inputs)
    nc = _get_program(P0, has_bias)

    common = {"wt": Wt}
    if has_bias:
        common["bh"] = bh
    in_maps = []
    for i in range(_NC):
        m = dict(common)
        m["x"] = np.ascontiguousarray(x[i * _SHARD:(i + 1) * _SHARD])
        in_maps.append(m)

    res = run_bass_kernel_spmd(nc, in_maps, core_ids=list(range(_NC)),
                               **(run_opts or {}))
    out = np.concatenate([r["o"] for r in res.results], axis=0)
    if b2 != 0.0:
        out = out + np.float32(b2)
    if run_opts:
        kernel.last_results = res
    return out.astype(np.float32)

